# revision 89
# baseline (speedup 1.0000x reference)
"""BKT model (MLP + per-chain 2-state HMM scan) on 8 Trainium2 NeuronCores.

Strategy
--------
Data-parallel over batch: core m handles batch rows [8m, 8m+8).

The reference scans T=1024 steps sequentially, but each of the 500 chains is
visited only ~2x per sequence (max 11).  Host-side we reorganize each core's
8*1024 timesteps by (chain, visit-index): the 4000 (batch,chain) segments are
pooled per core and sorted by visit count descending, so that in "round" r the
active segments are exactly a prefix.  Chains longer than VC=6 visits are
split: the second half is processed as TWO pseudo-segments with basis init
alphas e0/e1 (the recurrence is linear in alpha), and the host recombines
them with the parent's final alpha (a tiny extra "af" output) — this caps the
round count at 6 and removes a full MLP tile of padding.

Device (bf16 matmul path):
  Phase A (PE): MLP over the permuted rows: H^T = tanh(W1^T X^T + b1) in
                1024-column pair tiles; o^T = W2^T H^T via 16 col-tiled
                N=128 matmuls per tile directly into a [128,128] PSUM
                layout, so one small selector matmul transposes o into the
                segment-slot layout with no DMA hop.
  Phase B (DVE/ACT): per-visit HMM quantities in probability space
                (sigmoid instead of log-softmax; exact reformulation), with
                the o-independent parts precomputed mid-kernel.
  Phase C: <=6 sequential rounds; each round is a fully vectorized
                [128 x c_r] update of all active segments.  No gathers: all
                indexing is baked into the host-side permutation.

The kernel ships raw per-step [py0|py1] and the final alphas; the host does
the log/normalize and scatters back to (b, t) order.
"""

import numpy as np
import ml_dtypes

import concourse.bass as bass
import concourse.tile as tile
import concourse.mybir as mybir
from concourse import bacc
from concourse.bass_utils import run_bass_kernel_spmd

B, T, NF, NH, NK, NS = 64, 1024, 512, 512, 500, 2
NCORES, BPC, P = 8, 8, 128
F32 = mybir.dt.float32
F32R = mybir.dt.float32r
AF = mybir.ActivationFunctionType
OP = mybir.AluOpType
BF16 = mybir.dt.bfloat16
MM_BF16 = True  # bf16 matmul path (host-cast bf16 DMA) vs float32r


# ---------------------------------------------------------------------------
# host-side layout
# ---------------------------------------------------------------------------

VC = 6  # visit cap: chains longer than VC are split (B-half tracked as a
        # 2x2 matrix via two pseudo-segments with basis init alphas; the
        # host recombines using the parent's final alpha)


def _build_layout(kc):
    kc = np.asarray(kc)
    counts = np.zeros((B, NK), dtype=np.int64)
    for b in range(B):
        np.add.at(counts[b], kc[b].astype(np.int64), 1)
    assert counts.max() <= 2 * VC
    Vmax = int(min(VC, counts.max()))

    seg_meta = []  # per core: dict(seg_cnt, seg_kind, seg_parent, rank_of, nsplit)
    n_r = np.zeros((NCORES, Vmax), dtype=np.int64)
    NSEG = BPC * NK
    for m in range(NCORES):
        cnt = counts[m * BPC:(m + 1) * BPC].reshape(-1)
        cntA = np.minimum(cnt, VC)
        split_idx = np.nonzero(cnt > VC)[0]
        cntB = cnt[split_idx] - VC
        seg_cnt = np.concatenate([cntA, cntB, cntB])
        seg_kind = np.concatenate([
            np.zeros(NSEG, np.int64),
            np.full(len(split_idx), 1, np.int64),
            np.full(len(split_idx), 2, np.int64)])
        seg_parent = np.concatenate([np.arange(NSEG), split_idx, split_idx])
        order = np.argsort(-seg_cnt, kind="stable")
        rank_of = np.empty(len(seg_cnt), dtype=np.int64)
        rank_of[order] = np.arange(len(seg_cnt))
        for r in range(Vmax):
            n_r[m, r] = int((seg_cnt > r).sum())
        seg_meta.append(dict(seg_cnt=seg_cnt, seg_kind=seg_kind,
                             seg_parent=seg_parent, rank_of=rank_of,
                             order=order, split_idx=split_idx))

    c_r = np.maximum(1, (n_r.max(axis=0) + 127) // 128).astype(np.int64)
    Qc = int(c_r.sum())
    pad = (-Qc) % 4
    c_r[-1] += pad
    Qc += pad
    off_r = np.concatenate([[0], np.cumsum(c_r)[:-1]]).astype(np.int64)
    # chunks: unions of consecutive rounds whose end column is a multiple of 4
    # (so each 512-position matmul tile maps to exactly one chunk)
    chunks = []
    start_r = 0
    for r in range(Vmax):
        end_col = int(off_r[r] + c_r[r])
        if end_col % 4 == 0:
            col0 = int(off_r[start_r])
            chunks.append((start_r, r + 1, col0, end_col - col0))
            start_r = r + 1
    assert start_r == Vmax
    return dict(Vmax=Vmax, c_r=c_r, off_r=off_r, Qc=Qc, Q=128 * Qc,
                seg_meta=seg_meta, chunks=chunks)


def _build_host_tensors(inputs, lay):
    kc = np.asarray(inputs["kc"]).astype(np.int64)
    corr = np.asarray(inputs["corr"]).astype(np.int64)
    FM = np.ascontiguousarray(np.asarray(inputs["FM"], dtype=np.float32))
    obs = np.asarray(inputs["obs_logits"], dtype=np.float32)
    trans = np.asarray(inputs["trans_logits"], dtype=np.float32)
    init = np.asarray(inputs["init_logits"], dtype=np.float32)

    Vmax, c_r, off_r, Qc, Q = (lay["Vmax"], lay["c_r"], lay["off_r"],
                               lay["Qc"], lay["Q"])
    FMf = FM.reshape(-1, NF)

    per_core = []
    for m in range(NCORES):
        meta = lay["seg_meta"][m]
        rank_of, split_idx = meta["rank_of"], meta["split_idx"]
        NSEG = BPC * NK
        nsplit = len(split_idx)
        pos_of_split = {int(s): i for i, s in enumerate(split_idx)}

        perm = np.zeros(Q, dtype=np.int64)
        valid = np.zeros(Q, dtype=bool)
        merge = []  # (out_row, q0, q1, parent_rank)

        def slot(rank, r):
            return (off_r[r] + rank // 128) * 128 + (rank % 128)

        for bl in range(BPC):
            b = m * BPC + bl
            ord_t = np.argsort(kc[b], kind="stable")
            ch = kc[b][ord_t]
            visit = np.arange(T) - np.searchsorted(ch, ch)
            sid = bl * NK + ch
            lo = visit < VC
            q = slot(rank_of[sid[lo]], visit[lo])
            perm[q] = b * T + ord_t[lo]
            valid[q] = True
            for i in np.nonzero(~lo)[0]:
                pi = pos_of_split[int(sid[i])]
                vB = int(visit[i]) - VC
                q0 = int(slot(rank_of[NSEG + pi], vB))
                q1 = int(slot(rank_of[NSEG + nsplit + pi], vB))
                row = int(b * T + ord_t[i])
                perm[q0] = row
                perm[q1] = row
                merge.append((row, q0, q1, int(rank_of[sid[i]])))

        rows = perm
        ch_of_q = kc.reshape(-1)[rows]
        y_of_q = corr.reshape(-1)[rows]

        def plane(vals):
            return np.ascontiguousarray(vals.reshape(Qc, 128).T)

        og = obs[ch_of_q]
        tg = trans[ch_of_q]
        b2v = np.asarray(inputs["b2"], dtype=np.float32)
        og0 = np.concatenate([plane(og[:, 0, 0]), plane(og[:, 1, 0])], axis=1)
        # fold the +b2 of o into the og1 plane: ogd' = ogd - 2*b2[s]
        og1 = np.concatenate([plane(og[:, 0, 1] - 2.0 * b2v[0]),
                              plane(og[:, 1, 1] - 2.0 * b2v[1])], axis=1)
        tg0 = np.concatenate([plane(tg[:, 0, 0]), plane(tg[:, 0, 1])], axis=1)
        tg1 = np.concatenate([plane(tg[:, 1, 0]), plane(tg[:, 1, 1])], axis=1)
        sgn = plane((2.0 * y_of_q - 1.0).astype(np.float32))

        Sc = 32
        order = meta["order"]
        nseg_tot = len(order)
        assert nseg_tot <= 128 * Sc
        igf = np.zeros((128, 2 * Sc), dtype=np.float32)
        kind = meta["seg_kind"][order]
        seg_chain = meta["seg_parent"][order] % NK
        sl = np.arange(nseg_tot)
        ig0 = init[seg_chain, 0]
        ig1 = init[seg_chain, 1]
        # pseudo-segments start from basis alphas e0/e1 (expressed as a
        # +-20 logit gap -> exact 1/0 after the device's tanh transform)
        ig0 = np.where(kind == 1, 20.0, np.where(kind == 2, -20.0, ig0))
        ig1 = np.where(kind == 1, -20.0, np.where(kind == 2, 20.0, ig1))
        igf[sl % 128, sl // 128] = ig0
        igf[sl % 128, Sc + sl // 128] = ig1

        if MM_BF16:
            xT = np.ascontiguousarray(FMf[rows].T.astype(ml_dtypes.bfloat16))
        else:
            xT = np.ascontiguousarray(FMf[rows].T)

        per_core.append(dict(
            xT=xT,
            og0=np.ascontiguousarray(og0, dtype=np.float32),
            og1=np.ascontiguousarray(og1, dtype=np.float32),
            tg0=np.ascontiguousarray(tg0, dtype=np.float32),
            tg1=np.ascontiguousarray(tg1, dtype=np.float32),
            sgn=np.ascontiguousarray(sgn, dtype=np.float32),
            ig=igf,
            perm=perm, valid=valid, merge=merge,
        ))

    mdt = ml_dtypes.bfloat16 if MM_BF16 else np.float32
    w1 = np.ascontiguousarray(np.asarray(inputs["W1"], np.float32).astype(mdt))
    b1r = np.ascontiguousarray(
        np.asarray(inputs["b1"], np.float32).reshape(4, 128).T)
    # W2 padded to M=32 per k-chunk (zeros beyond the 2 real outputs) so the
    # col-tiled W2 matmuls initialize whole 32-partition PSUM groups.
    w2p = np.zeros((128, 4, 32), dtype=np.float32)
    w2p[:, :, 0:2] = (np.asarray(inputs["W2"], np.float32)
                      .reshape(4, 128, 2).transpose(1, 0, 2))
    w2r = np.ascontiguousarray(w2p.reshape(128, 128).astype(mdt))
    sel8 = np.zeros((128, 8), dtype=np.float32)
    for cc in range(4):
        for ss in range(2):
            sel8[32 * cc + ss, 2 * cc + ss] = 1.0
    sel8 = sel8.astype(mdt)
    shared = dict(w1=w1, b1r=b1r, w2r=w2r, sel8=sel8)
    return per_core, shared


# ---------------------------------------------------------------------------
# bass kernel
# ---------------------------------------------------------------------------

def _r2(ap, w2):
    """[128, 2*w] -> [128, 2, w] plane split."""
    return ap.rearrange("p (s w) -> p s w", s=2)


def _kernel_body(ctx, tc, lay, dram, repeat=1):
    nc = tc.nc
    Vmax, c_r, off_r, Qc, Q = (lay["Vmax"], lay["c_r"], lay["off_r"],
                               lay["Qc"], lay["Q"])
    NTILE = Q // 512
    cmax = int(max(c_r))

    singles = ctx.enter_context(tc.tile_pool(name="singles", bufs=1))
    xt_pool = ctx.enter_context(tc.tile_pool(name="xt", bufs=4))
    ht_pool = ctx.enter_context(tc.tile_pool(name="ht", bufs=2))
    sm_pool = ctx.enter_context(tc.tile_pool(name="sm", bufs=3))
    rpool = ctx.enter_context(tc.tile_pool(name="rounds", bufs=2))
    psum = ctx.enter_context(tc.tile_pool(name="psum", bufs=1, space="PSUM"))
    psum2 = ctx.enter_context(tc.tile_pool(name="psum2", bufs=2, space="PSUM"))

    for _rep in range(repeat):
        _kernel_rep(tc, lay, dram, singles, xt_pool, ht_pool, sm_pool, rpool,
                    psum, psum2)


def _kernel_rep(tc, lay, dram, singles, xt_pool, ht_pool, sm_pool, rpool,
                psum, psum2):
    nc = tc.nc
    Vmax, c_r, off_r, Qc, Q = (lay["Vmax"], lay["c_r"], lay["off_r"],
                               lay["Qc"], lay["Q"])
    NTILE = Q // 512
    cmax = int(max(c_r))
    chunks = lay["chunks"]

    # --- weights interleaved with the first x chunks on the SP ring ---
    MMDT = BF16 if MM_BF16 else F32R
    w1v = dram["w1"].rearrange("(k p) n -> p k n", p=P)
    w1sb = [singles.tile([P, 512], MMDT, tag=f"w1sb{k}", name=f"w1sb{k}")
            for k in range(4)]
    w2sb = singles.tile([P, 128], MMDT, tag="w2sb")
    b1sb = singles.tile([P, 4], F32, tag="b1sb")

    og0t = singles.tile([P, 2 * Qc], F32, tag="og0t")
    og1t = singles.tile([P, 2 * Qc], F32, tag="og1t")
    tg0t = singles.tile([P, 2 * Qc], F32, tag="tg0t")
    tg1t = singles.tile([P, 2 * Qc], F32, tag="tg1t")
    sgnt = singles.tile([P, Qc], F32, tag="sgnt")
    igt = singles.tile([P, 64], F32, tag="igt")

    # chunks >= TAIL0 share one py tile + one epilogue DMA (they complete
    # in the serial round tail; merging avoids serial small DMAs there)
    TAIL0 = 2 if len(chunks) > 3 else max(0, len(chunks) - 1)
    tail_col0 = chunks[TAIL0][2]
    py_ch = [singles.tile([P, 2 * w], F32, tag=f"py{ci}", name=f"py{ci}")
             for ci, (_, _, _, w) in enumerate(chunks[:TAIL0])]
    py_tail = singles.tile([P, 2 * (Qc - tail_col0)], F32, tag="pytail")
    xTv = dram["xT"].rearrange("(k p) q -> p k q", p=P)

    ocat_ch = [singles.tile([P, 2 * w], F32, tag=f"ocat{ci}", name=f"ocat{ci}")
               for ci, (_, _, _, w) in enumerate(chunks)]
    kpl_ch = [singles.tile([P, 8 * w], F32, tag=f"kpl{ci}", name=f"kpl{ci}")
              for ci, (_, _, _, w) in enumerate(chunks)]
    chunk_of_col = np.zeros(Qc, dtype=np.int64)
    for ci, (_, _, col0, w) in enumerate(chunks):
        chunk_of_col[col0:col0 + w] = ci

    state = dict(prev=None, pstride=32,
                 dout3=dram["out"].rearrange("p (s w) -> p s w", s=2))

    def emit_plane_loads():
        nc.scalar.dma_start(out=og0t, in_=dram["og0"])
        nc.scalar.dma_start(out=og1t, in_=dram["og1"])
        nc.gpsimd.dma_start(out=tg0t, in_=dram["tg0"])
        nc.gpsimd.dma_start(out=tg1t, in_=dram["tg1"])
        nc.gpsimd.dma_start(out=sgnt, in_=dram["sgn"])
        nc.scalar.dma_start(out=igt, in_=dram["ig"])
        # init state: a1 = sigmoid(ig1-ig0) = 0.5 + 0.5*tanh((ig1-ig0)/2)
        ad = sm_pool.tile([P, 32], F32, tag="ad", name="ad")
        nc.vector.tensor_sub(ad, igt[:, 32:64], igt[:, 0:32])
        th = sm_pool.tile([P, 32], F32, tag="th", name="th")
        nc.scalar.activation(out=th, in_=ad, func=AF.Tanh, scale=0.5)
        vinit = singles.tile([P, 64], F32, tag="vinit")
        nc.vector.tensor_scalar(out=vinit[:, 32:64], in0=th,
                                scalar1=0.5, scalar2=0.5,
                                op0=OP.mult, op1=OP.add)
        nc.vector.tensor_scalar(out=vinit[:, 0:32], in0=th,
                                scalar1=-0.5, scalar2=0.5,
                                op0=OP.mult, op1=OP.add)
        state["prev"] = vinit
        # precompute the o-independent plane parts for every chunk now (they
        # only need the just-loaded og/tg/sgn planes), so the per-chunk
        # critical chain after the MLP is as short as possible
        for ci in range(len(chunks)):
            build_planes_pre(ci)

    ogdc_ch = [singles.tile([P, 2 * w], F32, tag=f"ogd{ci}", name=f"ogd{ci}")
               for ci, (_, _, _, w) in enumerate(chunks)]
    sgt_ch = [singles.tile([P, 2 * w], F32, tag=f"sgt{ci}", name=f"sgt{ci}")
              for ci, (_, _, _, w) in enumerate(chunks)]
    tcm_ch = [singles.tile([P, 2 * w], F32, tag=f"tcm{ci}", name=f"tcm{ci}")
              for ci, (_, _, _, w) in enumerate(chunks)]

    def build_planes_pre(ci):
        r0, r1, col0, w = chunks[ci]
        ogdc, sgt, tcm = ogdc_ch[ci], sgt_ch[ci], tcm_ch[ci]
        nc.vector.tensor_tensor(out=_r2(ogdc, w),
                                in0=_r2(og1t, Qc)[:, :, col0:col0 + w],
                                in1=_r2(og0t, Qc)[:, :, col0:col0 + w],
                                op=OP.subtract)
        gt = sm_pool.tile([P, 2 * cmax], F32, tag="gt", name=f"gt{ci}")[:, 0:2 * w]
        nc.vector.tensor_tensor(out=_r2(gt, w),
                                in0=_r2(tg0t, Qc)[:, :, col0:col0 + w],
                                in1=_r2(tg1t, Qc)[:, :, col0:col0 + w],
                                op=OP.subtract)
        # sigmoid(x) = 0.5 + 0.5*tanh(x/2): keep ACT on the tanh table set
        nc.scalar.activation(out=sgt, in_=gt, func=AF.Tanh, scale=0.5)
        nc.vector.tensor_scalar(out=sgt, in0=sgt, scalar1=0.5, scalar2=0.5,
                                op0=OP.mult, op1=OP.add)
        nc.vector.tensor_scalar(out=tcm, in0=sgt,
                                scalar1=-1.0, scalar2=1.0,
                                op0=OP.mult, op1=OP.add)

    def build_planes(ci):
        r0, r1, col0, w = chunks[ci]
        oc = ocat_ch[ci]
        g = sm_pool.tile([P, 4 * cmax], F32, tag="g", name=f"g{ci}")[:, 0:4 * w]
        sg = sm_pool.tile([P, 4 * cmax], F32, tag="sg",
                          name=f"sg{ci}")[:, 0:4 * w]
        # g23 = ogd - 2*o  (ocat already holds 2*o)
        nc.vector.tensor_sub(g[:, 2 * w:4 * w], ogdc_ch[ci], oc)
        nc.vector.tensor_tensor(
            out=_r2(g[:, 0:2 * w], w), in0=_r2(g[:, 2 * w:4 * w], w),
            in1=sgnt[:, col0:col0 + w].unsqueeze(1).broadcast_to([P, 2, w]),
            op=OP.mult)
        nc.scalar.activation(out=sg, in_=g, func=AF.Tanh, scale=0.5)
        nc.vector.tensor_scalar(out=sg, in0=sg, scalar1=0.5, scalar2=0.5,
                                op0=OP.mult, op1=OP.add)
        # sg = [pe0,pe1 | p01,p11] (probabilities); sgt = T, tcm = 1-T
        kt = kpl_ch[ci]
        k4 = kt.rearrange("p (h q w) -> p h q w", h=2, q=4)
        nc.vector.tensor_scalar(out=k4[:, :, 2, :], in0=_r2(sg[:, 2 * w:4 * w], w),
                                scalar1=-1.0, scalar2=1.0,
                                op0=OP.mult, op1=OP.add)
        nc.vector.tensor_copy(out=k4[:, :, 3, :], in_=_r2(sg[:, 2 * w:4 * w], w))
        nc.vector.tensor_tensor(out=k4[:, :, 0, :], in0=_r2(sgt_ch[ci], w),
                                in1=_r2(sg[:, 0:2 * w], w), op=OP.mult)
        nc.vector.tensor_tensor(out=k4[:, :, 1, :], in0=_r2(tcm_ch[ci], w),
                                in1=_r2(sg[:, 0:2 * w], w), op=OP.mult)

    def run_rounds(ci):
        r0, r1, col0, w = chunks[ci]
        kt = kpl_ch[ci]
        k4v = kt.rearrange("p (j q w) -> p j q w", j=2, q=4)
        if ci >= TAIL0:
            pycol0 = tail_col0
            pyc = py_tail.rearrange("p (s w) -> p s w", s=2)
        else:
            pycol0 = col0
            pyc = py_ch[ci].rearrange("p (s w) -> p s w", s=2)
        for r in range(r0, r1):
            c = int(c_r[r]); off = int(off_r[r]); offl = off - col0
            prev, pstride = state["prev"], state["pstride"]
            u = rpool.tile([P, 8 * cmax], F32, tag="u", name=f"u{r}")[:, 0:8 * c]
            src = (prev[:, 0:2 * pstride].rearrange("p (j w) -> p j w", j=2)
                   [:, :, 0:c].unsqueeze(2).broadcast_to([P, 2, 4, c]))
            nc.vector.tensor_tensor(
                out=u.rearrange("p (j q w) -> p j q w", j=2, q=4),
                in0=src, in1=k4v[:, :, :, offl:offl + c], op=OP.mult)
            na = rpool.tile([P, 2 * cmax], F32, tag="na", name=f"na{r}")[:, 0:2 * c]
            nc.vector.tensor_add(na, u[:, 0:2 * c], u[:, 4 * c:6 * c])
            # py off the DVE alpha-chain: the Pool engine is otherwise idle
            nc.gpsimd.tensor_add(pyc[:, :, off - pycol0:off - pycol0 + c],
                                 _r2(u[:, 2 * c:4 * c], c),
                                 _r2(u[:, 6 * c:8 * c], c))
            v_t = rpool.tile([P, 2 * cmax], F32, tag="v2",
                             name=f"v2_{r}")[:, 0:2 * c]
            nc.vector.tensor_scalar_max(v_t, na, 1e-20)
            state["prev"], state["pstride"] = v_t, c

        # epilogue: stream raw [py0|py1] to DRAM (overlaps later tiles);
        # host takes log + normalizes.  Tail chunks flush as one DMA.
        if ci < TAIL0:
            nc.sync.dma_start(out=state["dout3"][:, :, col0:col0 + w],
                              in_=pyc)
        elif ci == len(chunks) - 1:
            # final alphas (for the host-side recombination of split
            # chains) go on the ACT ring so both end DMAs dispatch in
            # parallel
            nc.scalar.dma_start(out=dram["af"], in_=state["prev"])
            nc.sync.dma_start(out=state["dout3"][:, :, tail_col0:Qc],
                              in_=py_tail.rearrange("p (s w) -> p s w", s=2))

    next_chunk = [0]
    planes_built = [False] * len(chunks)
    cols_done = [0] * len(chunks)
    st8_q = []

    # host-provided selector: sel8[p, (c s)] = 1 iff p == 32c+s, so
    # pt = st8^T @ sel8 extracts+transposes the 8 live rows in one N=8 matmul
    sel8 = singles.tile([P, 8], MMDT, tag="sel8")

    def finish_tile(n, st8):
        # pt[x, (c s)] = st8[32c+s, x]: regular matmul st8^T @ sel8 — an
        # 8-column selector stream instead of a full 128-col transpose.
        # ocat stores 2*o directly (saves the o2c op in build_planes).
        pt = psum2.tile([P, 8], F32, tag="pt", name=f"pt{n}")
        nc.tensor.matmul(pt, lhsT=st8, rhs=sel8, start=True, stop=True)
        ci = int(chunk_of_col[4 * n])
        _, _, col0, w = chunks[ci]
        nc.vector.tensor_scalar_mul(
            _r2(ocat_ch[ci], w)[:, :, 4 * n - col0:4 * n - col0 + 4],
            pt.rearrange("p (c s) -> p s c", s=2), 2.0)
        cols_done[ci] += 4
        if cols_done[ci] == w:
            build_planes(ci)
            planes_built[ci] = True
            while (next_chunk[0] < len(chunks)
                   and planes_built[next_chunk[0]]):
                run_rounds(next_chunk[0])
                next_chunk[0] += 1

    # MLP over tile PAIRS (1024 q-columns) so each tanh covers FD=1024 with
    # a single per-partition bias (same m-chunk across the pair); a lone
    # trailing tile forms a 1-wide group.  Groups are processed with the
    # LAST group (tail-round columns) pulled two slots early so the serial
    # round chain starts before the MLP fully drains.
    groups = [(s, min(2, NTILE - s)) for s in range(0, NTILE, 2)]
    group_order = list(range(len(groups)))
    if len(group_order) >= 4:
        group_order = (group_order[:-3] + [group_order[-1]]
                       + group_order[-3:-1])

    def w2_finish(n, ht, t):
        # o^T for the 4 column-groups lands at partitions {32c, 32c+1} of a
        # [128,128] PSUM tile (16 N=128 matmuls via col tile_position, same
        # PE cycles as 4 N=512), so the partition rearrange needs no DMA
        # hop: lane-preserving DVE copy + one PE transpose.
        po = psum2.tile([P, 128], F32, tag="po", name=f"po{n}")
        for k in range(4):
            for c in range(4):
                nc.tensor.matmul(
                    po[32 * c:32 * c + 32, :],
                    lhsT=w2sb[:, 32 * k:32 * k + 32],
                    rhs=ht[:, k, 512 * t + 128 * c:512 * t + 128 * c + 128],
                    start=(k == 0), stop=(k == 3),
                    skip_group_check=True,
                    tile_position=(0, 32 * c))
        st8 = sm_pool.tile([P, 128], MMDT, tag="st8", name=f"st8{n}")
        nc.vector.tensor_copy(out=st8, in_=po)
        st8_q.append((n, st8))
        if len(st8_q) >= 2:
            finish_tile(*st8_q.pop(0))

    NG = len(group_order)
    for pi, gi in enumerate(group_order):
        s0, G = groups[gi]
        q0 = 512 * s0
        if pi == 1:
            emit_plane_loads()
        if pi == 0:
            # startup: weights on the scalar ring in parallel with x halves
            # on the SP ring
            for k in range(4):
                nc.scalar.dma_start(out=w1sb[k], in_=w1v[:, k, :])
            nc.scalar.dma_start(out=w2sb, in_=dram["w2r"])
            nc.scalar.dma_start(out=b1sb, in_=dram["b1r"])
        # per-t (FD-512 tanh) at the ends: lets the PE start on a half-load
        # at startup and overlaps W2(t0) with tanh(t1) in the tail
        per_t = pi <= 1 or pi == NG - 1
        xt = xt_pool.tile([P, 4, 1024], MMDT, tag="xt", name=f"xt{gi}")
        if pi == 0:
            # sel8 rides first on the SP ring (tiny), then a burst of small
            # matmuls keeps the PE activity monitor busy so the real MLP
            # stream starts at full clock instead of the throttled pstate
            nc.sync.dma_start(out=sel8, in_=dram["sel8"])
            warm = psum2.tile([P, 8], F32, tag="pt", name="warm")
            for i in range(25):
                nc.tensor.matmul(warm[0:8, :], lhsT=sel8, rhs=sel8,
                                 start=True, stop=True)
        if per_t:
            for t in range(G):
                nc.sync.dma_start(
                    out=xt[:, :, 512 * t:512 * t + 512],
                    in_=xTv[:, :, q0 + 512 * t:q0 + 512 * t + 512])
        else:
            nc.sync.dma_start(out=xt[:, :, 0:512 * G],
                              in_=xTv[:, :, q0:q0 + 512 * G])
        ht = ht_pool.tile([P, 4, 1024], MMDT, tag="ht", name=f"ht{gi}")
        if per_t:
            # W1+tanh for both halves first, W2 after: the PE FIFO then has
            # W1(t1) to chew on while tanh(t0) runs on ACT
            for t in range(G):
                phh = [psum.tile([P, 1024], F32, tag=f"hp{j}",
                                 name=f"hp{j}_{gi}_{t}") for j in range(2)]
                for m in range(4):
                    for k in range(4):
                        nc.tensor.matmul(
                            phh[m // 2][:, 512 * (m % 2):512 * (m % 2) + 512],
                            lhsT=w1sb[k][:, m * 128:(m + 1) * 128],
                            rhs=xt[:, k, 512 * t:512 * t + 512],
                            start=(k == 0), stop=(k == 3))
                for m in range(4):
                    nc.scalar.activation(
                        out=ht[:, m, 512 * t:512 * t + 512],
                        in_=phh[m // 2][:, 512 * (m % 2):512 * (m % 2) + 512],
                        func=AF.Tanh, bias=b1sb[:, m:m + 1], scale=1.0)
            for t in range(G):
                w2_finish(s0 + t, ht, t)
            continue
        for m in range(4):
            ph = psum.tile([P, 1024], F32, tag=f"hp{m % 2}",
                           name=f"h{m}_{gi}")
            for t in range(G):
                for k in range(4):
                    nc.tensor.matmul(
                        ph[:, 512 * t:512 * t + 512],
                        lhsT=w1sb[k][:, m * 128:(m + 1) * 128],
                        rhs=xt[:, k, 512 * t:512 * t + 512],
                        start=(k == 0), stop=(k == 3))
            nc.scalar.activation(out=ht[:, m, 0:512 * G],
                                 in_=ph[:, 0:512 * G], func=AF.Tanh,
                                 bias=b1sb[:, m:m + 1], scale=1.0)
        for t in range(G):
            w2_finish(s0 + t, ht, t)

    while st8_q:
        finish_tile(*st8_q.pop(0))
    while next_chunk[0] < len(chunks):
        if not planes_built[next_chunk[0]]:
            build_planes(next_chunk[0])
            planes_built[next_chunk[0]] = True
        run_rounds(next_chunk[0])
        next_chunk[0] += 1


def _build_nc(lay, repeat=1):
    from contextlib import ExitStack
    nc = bacc.Bacc("TRN2", target_bir_lowering=False, debug=False,
                   num_devices=NCORES)
    Qc, Q = lay["Qc"], lay["Q"]
    dram = {}
    def din(name, shape, dt=F32):
        dram[name] = nc.dram_tensor(name, shape, dt, kind="ExternalInput").ap()
    mmin = BF16 if MM_BF16 else F32R
    din("xT", [NF, Q], mmin)
    din("w1", [NF, NH], mmin)
    din("b1r", [P, 4])
    din("sel8", [P, 8], mmin)
    din("w2r", [P, 128], mmin)
    din("og0", [P, 2 * Qc])
    din("og1", [P, 2 * Qc])
    din("tg0", [P, 2 * Qc])
    din("tg1", [P, 2 * Qc])
    din("sgn", [P, Qc])
    din("ig", [P, 64])
    dram["out"] = nc.dram_tensor("out", [P, 2 * Qc], F32,
                                 kind="ExternalOutput").ap()
    dram["af"] = nc.dram_tensor("af", [P, 2 * int(lay["c_r"][-1])], F32,
                                kind="ExternalOutput").ap()
    with tile.TileContext(nc) as tc:
        with ExitStack() as ctx:
            _kernel_body(ctx, tc, lay, dram, repeat=repeat)
    nc.compile()
    return nc


_NC_CACHE = {}


def _get_nc(lay):
    key = tuple(int(x) for x in lay["c_r"])
    if key not in _NC_CACHE:
        _NC_CACHE[key] = _build_nc(lay)
    return _NC_CACHE[key]


# ---------------------------------------------------------------------------
# entry point
# ---------------------------------------------------------------------------

def _feed(c, shared):
    return dict(
        xT=c["xT"], w1=shared["w1"], b1r=shared["b1r"], w2r=shared["w2r"],
        sel8=shared["sel8"], og0=c["og0"], og1=c["og1"], tg0=c["tg0"],
        tg1=c["tg1"], sgn=c["sgn"], ig=c["ig"])


def _unpack_core(out, OUT, AF, lay, c):
    """Scatter one core's raw [py0|py1] planes into out[B*T, 2] as
    normalized log-probs, recombining split-chain visits with the parent's
    final alpha."""
    Qc, Q = lay["Qc"], lay["Q"]
    J = np.arange(Q) // 128
    p = np.arange(Q) % 128
    g = c["perm"]; v = c["valid"]
    py0 = OUT[p[v], J[v]].astype(np.float64)
    py1 = OUT[p[v], Qc + J[v]].astype(np.float64)
    s = np.log(py0 + py1)
    out[g[v], 0] = np.log(py0) - s
    out[g[v], 1] = np.log(py1) - s
    if c["merge"]:
        mg = np.asarray(c["merge"], dtype=np.int64)
        rows, q0, q1, prank = mg[:, 0], mg[:, 1], mg[:, 2], mg[:, 3]
        assert prank.max() < 128
        c_last = int(lay["c_r"][-1])
        a0 = AF[prank, 0].astype(np.float64)
        a1 = AF[prank, c_last].astype(np.float64)
        py0 = (OUT[q0 % 128, q0 // 128] * a0
               + OUT[q1 % 128, q1 // 128] * a1)
        py1 = (OUT[q0 % 128, Qc + q0 // 128] * a0
               + OUT[q1 % 128, Qc + q1 // 128] * a1)
        s = np.log(py0 + py1)
        out[rows, 0] = np.log(py0) - s
        out[rows, 1] = np.log(py1) - s


def kernel(corr, kc, FM, W1, b1, W2, b2, trans_logits, obs_logits, init_logits,
           _want_results_only=True, _trace=False):
    inputs = dict(corr=corr, kc=kc, FM=FM, W1=W1, b1=b1, W2=W2, b2=b2,
                  trans_logits=trans_logits, obs_logits=obs_logits,
                  init_logits=init_logits)
    lay = _build_layout(kc)
    nc = _get_nc(lay)
    per_core, shared = _build_host_tensors(inputs, lay)

    in_maps = [_feed(per_core[m], shared) for m in range(NCORES)]

    res = run_bass_kernel_spmd(nc, in_maps, core_ids=list(range(NCORES)),
                               trace=_trace)

    out = np.zeros((B * T, 2), dtype=np.float32)
    for m in range(NCORES):
        _unpack_core(out, res.results[m]["out"], res.results[m]["af"],
                     lay, per_core[m])
    out = out.reshape(B, T, 2)
    if _want_results_only:
        return out
    return out, res

